# revision 14
# baseline (speedup 1.0000x reference)
"""MiniGPT (B=2,S=2048,D=1024,H=16,F=4096,L=4,V=50257) on 8 Trainium2 cores.

Sharding: sequence-parallel. Core c owns rows of batch c//4 at positions
[512*(c%4), 512*(c%4)+512).  Per layer each core computes QKV/attention/FFN
for its 512 rows; K/V are exchanged with one AllGather per layer within each
batch group ([[0-3],[4-7]]).  Attention is computed dense (all 16 key chunks)
with per-core 0/1 masks (preloaded once into SBUF) so the SPMD program is
uniform.  The LM head needs no collective: each core computes logits for its
own 512 rows over the FULL vocab (same FLOPs as a vocab-sharded head over all
rows), streaming the head weights from HBM in a host-pretiled layout, and
emits logits^T in bf16; the host reassembles [B,S,V] f32.  All matmuls run in
bf16 with fp32 accumulation.
"""

import json
import math

import numpy as np
import ml_dtypes

# ---------------------------------------------------------------------------
# Workaround for this container's walrus build: it can lower at most ONE sem
# wait per instruction ("Too many sync wait commands" otherwise), while Tile
# attaches several waits to one instruction.  Rewrite the BIR JSON just before
# it reaches walrus: any instruction carrying N>1 waits gets N-1 preceding
# single-wait NoOps on the same engine (identical semantics).
# ---------------------------------------------------------------------------
import concourse.bass_utils as bass_utils
import concourse.bass2jax as bass2jax

_orig_compile_bir_kernel = bass_utils.compile_bir_kernel


def _split_multi_waits(bir_json):
    m = json.loads(bir_json)
    changed = False
    for fn in m.get("functions", []):
        for blk in fn.get("blocks", []):
            new_insts = []
            for ins in blk.get("instructions", []):
                si = ins.get("sync_info")
                waits = (si or {}).get("on_wait") or []
                if len(waits) > 1:
                    changed = True
                    for i, w in enumerate(waits[:-1]):
                        new_insts.append(
                            {
                                "debug": ins.get("debug"),
                                "engine": ins["engine"],
                                "ins": [],
                                "name": f"{ins['name']}-sw{i}",
                                "opcode": "NoOp",
                                "outs": [],
                                "sync_info": {"on_update": [], "on_wait": [w]},
                                "text_hint": "split_wait",
                            }
                        )
                    si["on_wait"] = [waits[-1]]
                new_insts.append(ins)
            blk["instructions"] = new_insts
    if not changed:
        return bir_json
    return json.dumps(m).encode()


def _patched_compile_bir_kernel(bir_json, tmpdir, neff_name="file.neff"):
    if isinstance(bir_json, str):
        bir_json = bir_json.encode()
    return _orig_compile_bir_kernel(_split_multi_waits(bir_json), tmpdir, neff_name)


if bass_utils.compile_bir_kernel is _orig_compile_bir_kernel:
    bass_utils.compile_bir_kernel = _patched_compile_bir_kernel
    bass2jax.compile_bir_kernel = _patched_compile_bir_kernel

import concourse.bass as bass
import concourse.tile as tile
import concourse.mybir as mybir
from concourse.masks import make_identity

# ---------------------------------------------------------------------------
# Model / sharding constants
# ---------------------------------------------------------------------------
NC = 8                  # cores
B, S, D, H, DK, F, L, V = 2, 2048, 1024, 16, 64, 4096, 4, 50257
P = 128
RPC = 512               # rows per core
NRS = RPC // P          # 4 row subtiles
NJ = D // P             # 8 d-tiles
NHP = H // 2            # 8 head pairs
NKC = 16                # key chunks of 128 (per batch group)
NFT = F // P            # 32 ffn tiles
VP2 = 50304             # padded vocab (393 * 128)
NVT2 = VP2 // P         # 393 vocab subtiles
BF16 = mybir.dt.bfloat16
F32 = mybir.dt.float32
F16 = mybir.dt.float16
I32 = mybir.dt.int32
EXP_SCALE = 1.0 / math.sqrt(DK)


def _ln_chunk(nc, sb3, sbw, ps_t, x, g_str, b_str, ident, eps, out_T, rs):
    """LayerNorm the 128 rows in x ([P, D] f32) and write the transposed
    normalized result into out_T[:, :, rs*P:(rs+1)*P]."""
    if True:
        stats = sb3.tile([P, 2, 6], F32, tag="ln_stats")
        xg = x.rearrange("p (g d) -> p g d", g=2)
        for gi in range(2):
            nc.vector.bn_stats(out=stats[:, gi, :], in_=xg[:, gi, :])
        mv = sb3.tile([P, 2], F32, tag="ln_mv")
        nc.vector.bn_aggr(out=mv[:], in_=stats[:])
        rstd = sb3.tile([P, 1], F32, tag="ln_rstd")
        nc.scalar.activation(out=rstd[:], in_=mv[:, 1:2],
                             func=mybir.ActivationFunctionType.Sqrt,
                             bias=eps[:], scale=1.0)
        nc.vector.reciprocal(out=rstd[:], in_=rstd[:])
        hnc = sbw.tile([P, D], BF16, tag="ln_hnc")
        nc.vector.tensor_scalar(out=hnc[:], in0=x, scalar1=mv[:, 0:1], scalar2=rstd[:],
                                op0=mybir.AluOpType.subtract, op1=mybir.AluOpType.mult)
        for j in range(NJ):
            pt = ps_t.tile([P, P], BF16, tag="ps_t")
            nc.tensor.transpose(pt[:], hnc[:, j * P:(j + 1) * P], ident[:])
            nc.vector.tensor_scalar(out=out_T[:, j, rs * P:(rs + 1) * P], in0=pt[:],
                                    scalar1=g_str[:, j:j + 1], scalar2=b_str[:, j:j + 1],
                                    op0=mybir.AluOpType.mult, op1=mybir.AluOpType.add)


def _ln_into_transposed(nc, sb3, sbw, ps_t, h_sb, rs_range, g_str, b_str, ident, eps, out_T):
    for rs in rs_range:
        _ln_chunk(nc, sb3, sbw, ps_t, h_sb[:, rs, :], g_str, b_str, ident, eps, out_T, rs)


def build(n_layers=L, emit="logits"):
    """Build the SPMD Bass module.  emit: "logits" (full model) or "hidden"
    (stop after n_layers, output h [P, NRS, D] f32 for debugging)."""
    nc = bass.Bass(num_devices=NC)

    # ---- inputs (per core) ----
    emb16 = nc.dram_tensor("emb16", [V, D], F16, kind="ExternalInput")
    idx_pa = nc.dram_tensor("idx_pa", [P, NRS], I32, kind="ExternalInput")
    idx_full = nc.dram_tensor("idx_full", [P, NKC], I32, kind="ExternalInput")
    pe_full = nc.dram_tensor("pe_full", [P, NKC, D], F32, kind="ExternalInput")
    pe_pa = nc.dram_tensor("pe_pa", [P, NRS, D], F32, kind="ExternalInput")
    wq = nc.dram_tensor("wq", [L, D, D], BF16, kind="ExternalInput")
    wk = nc.dram_tensor("wk", [L, D, D], BF16, kind="ExternalInput")
    wv = nc.dram_tensor("wv", [L, D, D], BF16, kind="ExternalInput")
    wo = nc.dram_tensor("wo", [L, D, D], BF16, kind="ExternalInput")
    w1 = nc.dram_tensor("w1", [L, D, F], BF16, kind="ExternalInput")
    w2 = nc.dram_tensor("w2", [L, F, D], BF16, kind="ExternalInput")
    ln1g = nc.dram_tensor("ln1g", [L, D], F32, kind="ExternalInput")
    ln1b = nc.dram_tensor("ln1b", [L, D], F32, kind="ExternalInput")
    ln2g = nc.dram_tensor("ln2g", [L, D], F32, kind="ExternalInput")
    ln2b = nc.dram_tensor("ln2b", [L, D], F32, kind="ExternalInput")
    lnfg = nc.dram_tensor("lnfg", [D], F32, kind="ExternalInput")
    lnfb = nc.dram_tensor("lnfb", [D], F32, kind="ExternalInput")
    b1s = nc.dram_tensor("b1s", [L, P, NFT], F32, kind="ExternalInput")
    b2 = nc.dram_tensor("b2", [L, D], F32, kind="ExternalInput")
    masks = nc.dram_tensor("masks", [P, NKC, RPC], BF16, kind="ExternalInput")
    hw = nc.dram_tensor("hw", [NVT2, P, NJ * P], BF16, kind="ExternalInput")
    hbs = nc.dram_tensor("hbs", [P, NVT2], F32, kind="ExternalInput")

    if emit == "hidden":
        h_out = nc.dram_tensor("h_out", [P, NRS, D], F32, kind="ExternalOutput")
    else:
        logitsT = nc.dram_tensor("logitsT", [VP2, RPC], BF16, kind="ExternalOutput")

    kv_groups = [[0, 1, 2, 3], [4, 5, 6, 7]]

    with tile.TileContext(nc) as tc:
        with (
            tc.tile_pool(name="singles", bufs=1) as singles,
            tc.tile_pool(name="h", bufs=1) as hpool,
            tc.tile_pool(name="params", bufs=1) as params,
            tc.tile_pool(name="sbw", bufs=2) as sbw,
            tc.tile_pool(name="sb3", bufs=3) as sb3,
            tc.tile_pool(name="dram", bufs=2, space="DRAM") as dram,
        ):
            ident = singles.tile([P, P], BF16)
            make_identity(nc, ident[:])
            eps = singles.tile([P, 1], F32)
            nc.vector.memset(eps[:], 1e-5)
            ones64 = singles.tile([1, 64], F32)
            nc.vector.memset(ones64[:], 1.0)
            h_sb = hpool.tile([P, NRS, D], F32)

            # ---- embedding gather + positional encoding ----
            # (tiny index DMAs go first so the gathers are not queued behind
            # the 2MB mask load)
            idx_sb = sb3.tile([P, NRS], I32, tag="idx")
            nc.sync.dma_start(idx_sb[:], idx_pa.ap())
            mask_sb = singles.tile([P, NKC, RPC], BF16)
            nc.sync.dma_start(mask_sb[:], masks.ap())
            for rs in range(NRS):
                eg = sbw.tile([P, D], F16, tag="embg")
                nc.gpsimd.indirect_dma_start(
                    out=eg[:], out_offset=None, in_=emb16.ap(),
                    in_offset=bass.IndirectOffsetOnAxis(ap=idx_sb[:, rs:rs + 1], axis=0),
                )
                pe = sbw.tile([P, D], F32, tag="peg")
                nc.sync.dma_start(pe[:], pe_pa.ap()[:, rs, :])
                nc.vector.tensor_copy(out=h_sb[:, rs, :], in_=eg[:])
                nc.vector.tensor_add(out=h_sb[:, rs, :], in0=h_sb[:, rs, :], in1=pe[:])

            with tc.tile_pool(name="sb1", bufs=1) as sb1:
                # ---- layer 0: every core computes K/V for ALL 2048 rows of
                # its batch straight from the (replicated) embeddings — no
                # AllGather needed for layer 0. ----
                kv0 = dram.tile([4, 2, P, NRS * D], BF16, tag="kv_all")
                g10 = params.tile([P, NJ], F32, tag="g1")
                b10 = params.tile([P, NJ], F32, tag="b1t")
                nc.sync.dma_start(g10[:], ln1g.ap()[0].rearrange("(j p) -> p j", p=P))
                nc.sync.dma_start(b10[:], ln1b.ap()[0].rearrange("(j p) -> p j", p=P))
                idxf_sb = sb3.tile([P, NKC], I32, tag="idx")
                nc.sync.dma_start(idxf_sb[:], idx_full.ap())
                with (
                    tc.tile_pool(name="ps_mm0", bufs=3, space="PSUM") as ps_mm0,
                    tc.tile_pool(name="ps_t0", bufs=2, space="PSUM") as ps_t0,
                ):
                    wk0 = sbw.tile([P, NJ, D], BF16, tag="w_dd")
                    nc.sync.dma_start(wk0[:], wk.ap()[0].rearrange("(j p) n -> p j n", p=P))
                    wv0 = sbw.tile([P, NJ, D], BF16, tag="w_dd")
                    nc.sync.dma_start(wv0[:], wv.ap()[0].rearrange("(j p) n -> p j n", p=P))
                    for b4 in range(4):
                        hnTg = sb1.tile([P, NJ, RPC], BF16, tag="hnT")
                        for rs in range(NRS):
                            rc = b4 * NRS + rs
                            eg = sbw.tile([P, D], F16, tag="embg")
                            nc.gpsimd.indirect_dma_start(
                                out=eg[:], out_offset=None, in_=emb16.ap(),
                                in_offset=bass.IndirectOffsetOnAxis(
                                    ap=idxf_sb[:, rc:rc + 1], axis=0),
                            )
                            pef = sbw.tile([P, D], F32, tag="peg")
                            nc.sync.dma_start(pef[:], pe_full.ap()[:, rc, :])
                            hch = sb3.tile([P, D], F32, tag="hch")
                            nc.vector.tensor_copy(out=hch[:], in_=eg[:])
                            nc.vector.tensor_add(out=hch[:], in0=hch[:], in1=pef[:])
                            _ln_chunk(nc, sb3, sbw, ps_t0, hch[:], g10, b10,
                                      ident, eps, hnTg, rs)
                        ktg = sb1.tile([P, NHP, RPC], BF16, tag="kt")
                        for hp in range(NHP):
                            pq = ps_mm0.tile([P, RPC], F32, tag="ps_mm")
                            for j in range(NJ):
                                nc.tensor.matmul(pq[:], wk0[:, j, hp * P:(hp + 1) * P],
                                                 hnTg[:, j, :], start=(j == 0),
                                                 stop=(j == NJ - 1))
                            nc.scalar.activation(out=ktg[:, hp, :], in_=pq[:],
                                                 func=mybir.ActivationFunctionType.Copy)
                        nc.sync.dma_start(kv0[b4, 0], ktg[:].rearrange("p a b -> p (a b)"))
                        vg = sb1.tile([P, NRS, D], BF16, tag="vown")
                        for rs in range(NRS):
                            for nh in range(2):
                                pv = ps_mm0.tile([P, RPC], F32, tag="ps_mm")
                                for j in range(NJ):
                                    nc.tensor.matmul(pv[:], hnTg[:, j, rs * P:(rs + 1) * P],
                                                     wv0[:, j, nh * 512:(nh + 1) * 512],
                                                     start=(j == 0), stop=(j == NJ - 1))
                                nc.scalar.activation(out=vg[:, rs, nh * 512:(nh + 1) * 512],
                                                     in_=pv[:],
                                                     func=mybir.ActivationFunctionType.Copy)
                        nc.sync.dma_start(kv0[b4, 1], vg[:].rearrange("p a b -> p (a b)"))
                    # own rows: LN1 + Q projection for layer 0
                    hnT0 = sb1.tile([P, NJ, RPC], BF16, tag="hnT")
                    _ln_into_transposed(nc, sb3, sbw, ps_t0, h_sb, range(NRS), g10, b10,
                                        ident, eps, hnT0)
                    qt0 = sb1.tile([P, NHP, RPC], BF16, tag="qt")
                    wq0 = sbw.tile([P, NJ, D], BF16, tag="w_dd")
                    nc.sync.dma_start(wq0[:], wq.ap()[0].rearrange("(j p) n -> p j n", p=P))
                    for hp in range(NHP):
                        pq = ps_mm0.tile([P, RPC], F32, tag="ps_mm")
                        for j in range(NJ):
                            nc.tensor.matmul(pq[:], wq0[:, j, hp * P:(hp + 1) * P],
                                             hnT0[:, j, :], start=(j == 0), stop=(j == NJ - 1))
                        nc.scalar.activation(out=qt0[:, hp, :], in_=pq[:],
                                             func=mybir.ActivationFunctionType.Copy)

                for l in range(n_layers):
                    # ---- layer parameter tiles ----
                    g2 = params.tile([P, NJ], F32, tag="g2")
                    b2t = params.tile([P, NJ], F32, tag="b2t")
                    nc.sync.dma_start(g2[:], ln2g.ap()[l].rearrange("(j p) -> p j", p=P))
                    nc.sync.dma_start(b2t[:], ln2b.ap()[l].rearrange("(j p) -> p j", p=P))
                    b1v = params.tile([P, NFT], F32, tag="b1v")
                    nc.sync.dma_start(b1v[:], b1s.ap()[l])
                    b2bc = params.tile([P, D], F32, tag="b2bc")
                    nc.sync.dma_start(
                        b2bc[:],
                        bass.AP(tensor=b2.ap().tensor, offset=l * D, ap=[[0, P], [1, D]]),
                    )

                    if l == 0:
                        kv_all = kv0
                        qt_sb = qt0
                    else:
                        g1 = params.tile([P, NJ], F32, tag="g1")
                        b1t = params.tile([P, NJ], F32, tag="b1t")
                        nc.sync.dma_start(g1[:], ln1g.ap()[l].rearrange("(j p) -> p j", p=P))
                        nc.sync.dma_start(b1t[:], ln1b.ap()[l].rearrange("(j p) -> p j", p=P))

                        kv_in = dram.tile([2, P, NRS * D], BF16, tag="kv_in")
                        kv_all = dram.tile([4, 2, P, NRS * D], BF16, tag="kv_all")

                        with (
                            tc.tile_pool(name="ps_mm", bufs=3, space="PSUM") as ps_mm,
                            tc.tile_pool(name="ps_t", bufs=2, space="PSUM") as ps_t,
                        ):
                            # ---- ln1 -> hnT ----
                            hnT = sb1.tile([P, NJ, RPC], BF16, tag="hnT")
                            _ln_into_transposed(nc, sb3, sbw, ps_t, h_sb, range(NRS), g1, b1t,
                                                ident, eps, hnT)

                            # ---- K and V projections first (feed the AllGather) ----
                            kt_sb = sb1.tile([P, NHP, RPC], BF16, tag="kt")
                            wk_sb = sbw.tile([P, NJ, D], BF16, tag="w_dd")
                            nc.sync.dma_start(wk_sb[:], wk.ap()[l].rearrange("(j p) n -> p j n", p=P))
                            for hp in range(NHP):
                                pq = ps_mm.tile([P, RPC], F32, tag="ps_mm")
                                for j in range(NJ):
                                    nc.tensor.matmul(pq[:], wk_sb[:, j, hp * P:(hp + 1) * P],
                                                     hnT[:, j, :], start=(j == 0), stop=(j == NJ - 1))
                                nc.scalar.activation(out=kt_sb[:, hp, :], in_=pq[:],
                                                     func=mybir.ActivationFunctionType.Copy)
                            nc.sync.dma_start(kv_in[0], kt_sb[:].rearrange("p a b -> p (a b)"))

                            v_sb = sb1.tile([P, NRS, D], BF16, tag="vown")
                            wv_sb = sbw.tile([P, NJ, D], BF16, tag="w_dd")
                            nc.sync.dma_start(wv_sb[:], wv.ap()[l].rearrange("(j p) n -> p j n", p=P))
                            for rs in range(NRS):
                                for nh in range(2):
                                    pv = ps_mm.tile([P, RPC], F32, tag="ps_mm")
                                    for j in range(NJ):
                                        nc.tensor.matmul(pv[:], hnT[:, j, rs * P:(rs + 1) * P],
                                                         wv_sb[:, j, nh * 512:(nh + 1) * 512],
                                                         start=(j == 0), stop=(j == NJ - 1))
                                    nc.scalar.activation(out=v_sb[:, rs, nh * 512:(nh + 1) * 512],
                                                         in_=pv[:],
                                                         func=mybir.ActivationFunctionType.Copy)
                            nc.sync.dma_start(kv_in[1], v_sb[:].rearrange("p a b -> p (a b)"))

                        nc.gpsimd.collective_compute(
                            "AllGather", mybir.AluOpType.bypass, replica_groups=kv_groups,
                            ins=[kv_in.opt()], outs=[kv_all.opt()],
                        )

                        with (
                            tc.tile_pool(name="ps_mm", bufs=3, space="PSUM") as ps_mm,
                        ):
                            # ---- Q projection (overlaps the AllGather) ----
                            qt_sb = sb1.tile([P, NHP, RPC], BF16, tag="qt")
                            wq_sb = sbw.tile([P, NJ, D], BF16, tag="w_dd")
                            nc.sync.dma_start(wq_sb[:], wq.ap()[l].rearrange("(j p) n -> p j n", p=P))
                            for hp in range(NHP):
                                pq = ps_mm.tile([P, RPC], F32, tag="ps_mm")
                                for j in range(NJ):
                                    nc.tensor.matmul(pq[:], wq_sb[:, j, hp * P:(hp + 1) * P],
                                                     hnT[:, j, :], start=(j == 0), stop=(j == NJ - 1))
                                nc.scalar.activation(out=qt_sb[:, hp, :], in_=pq[:],
                                                     func=mybir.ActivationFunctionType.Copy)

                    kv_k = kv_all[:].rearrange("g t p (hp r) -> g t p hp r", hp=NHP)
                    kv_v = kv_all[:].rearrange("g t p (rs hh d) -> g t p rs hh d", rs=NRS, hh=H)

                    # ---- attention (dense over 16 key chunks, masked) ----
                    attnT = sb1.tile([P, NHP, RPC], BF16, tag="attnT")
                    with (
                        tc.tile_pool(name="ps_s", bufs=2, space="PSUM") as ps_s,
                        tc.tile_pool(name="ps_pv", bufs=4, space="PSUM") as ps_pv,
                        tc.tile_pool(name="ps_bc", bufs=1, space="PSUM") as ps_bc,
                    ):
                        for hg in range(4):
                            pvs = [ps_pv.tile([65, RPC], F32, tag="ps_pv", name=f"pv{hg}_{i}") for i in range(4)]
                            for kc in range(NKC):
                                gr, rs = kc // 4, kc % 4
                                ktt = sb3.tile([P, 2, P], BF16, tag="ktt")
                                nc.sync.dma_start(
                                    ktt[:], kv_k[gr, 0, :, 2 * hg:2 * hg + 2, rs * P:(rs + 1) * P])
                                vat = sb3.tile([P, 4, 65], BF16, tag="vat")
                                nc.vector.memset(vat[:, :, 64:65], 1.0)
                                nc.sync.dma_start(
                                    vat[:, :, 0:64], kv_v[gr, 1, :, rs, 4 * hg:4 * hg + 4, :])
                                for hi in range(4):
                                    h_ = 4 * hg + hi
                                    hp, o = h_ // 2, (h_ % 2) * 64
                                    pss = ps_s.tile([P, RPC], F32, tag="ps_s")
                                    nc.tensor.matmul(pss[:], ktt[o:o + 64, hi // 2, :],
                                                     qt_sb[o:o + 64, hp, :], start=True, stop=True)
                                    et = sb3.tile([P, RPC], BF16, tag="et")
                                    nc.scalar.activation(out=et[:], in_=pss[:],
                                                         func=mybir.ActivationFunctionType.Exp,
                                                         scale=EXP_SCALE)
                                    nc.vector.tensor_mul(out=et[:], in0=et[:], in1=mask_sb[:, kc, :])
                                    nc.tensor.matmul(pvs[hi][:], vat[:, hi, :], et[:],
                                                     start=(kc == 0), stop=(kc == NKC - 1))
                            for hi in range(4):
                                h_ = 4 * hg + hi
                                hp, o = h_ // 2, (h_ % 2) * 64
                                rec = sb3.tile([1, RPC], F32, tag="rec")
                                nc.vector.reciprocal(out=rec[:], in_=pvs[hi][64:65, :])
                                pbc = ps_bc.tile([64, RPC], F32, tag="ps_bc")
                                nc.tensor.matmul(pbc[:], ones64[:], rec[:], start=True, stop=True)
                                bcs = sb3.tile([64, RPC], F32, tag="bcs")
                                nc.scalar.activation(out=bcs[:], in_=pbc[:],
                                                     func=mybir.ActivationFunctionType.Copy)
                                nc.vector.tensor_mul(out=attnT[o:o + 64, hp, :],
                                                     in0=pvs[hi][0:64, :], in1=bcs[:])

                    with (
                        tc.tile_pool(name="ps_mm", bufs=3, space="PSUM") as ps_mm,
                        tc.tile_pool(name="ps_t", bufs=2, space="PSUM") as ps_t,
                    ):
                        # ---- attn output projection + residual ----
                        wo_sb = sbw.tile([P, NJ, D], BF16, tag="w_dd")
                        nc.sync.dma_start(wo_sb[:], wo.ap()[l].rearrange("(j p) n -> p j n", p=P))
                        for rs in range(NRS):
                            for nh in range(2):
                                po = ps_mm.tile([P, RPC], F32, tag="ps_mm")
                                for j in range(NJ):
                                    nc.tensor.matmul(po[:], attnT[:, j, rs * P:(rs + 1) * P],
                                                     wo_sb[:, j, nh * 512:(nh + 1) * 512],
                                                     start=(j == 0), stop=(j == NJ - 1))
                                nc.vector.tensor_add(out=h_sb[:, rs, nh * 512:(nh + 1) * 512],
                                                     in0=h_sb[:, rs, nh * 512:(nh + 1) * 512],
                                                     in1=po[:])

                        # ---- FFN ----
                        hnT2 = sb1.tile([P, NJ, RPC], BF16, tag="hnT")
                        _ln_into_transposed(nc, sb3, sbw, ps_t, h_sb, range(NRS), g2, b2t,
                                            ident, eps, hnT2)
                        aT = sb1.tile([P, NFT, RPC], BF16, tag="aT")
                        for q in range(4):
                            w1q = sbw.tile([P, NJ, D], BF16, tag="w_dd")
                            nc.sync.dma_start(
                                w1q[:],
                                wa_slice(w1.ap()[l], q))
                            for fl in range(8):
                                ft = q * 8 + fl
                                pa = ps_mm.tile([P, RPC], F32, tag="ps_mm")
                                for j in range(NJ):
                                    nc.tensor.matmul(pa[:], w1q[:, j, fl * P:(fl + 1) * P],
                                                     hnT2[:, j, :], start=(j == 0), stop=(j == NJ - 1))
                                nc.scalar.activation(out=aT[:, ft, :], in_=pa[:],
                                                     func=mybir.ActivationFunctionType.Relu,
                                                     bias=b1v[:, ft:ft + 1], scale=1.0)
                        for nq in range(4):
                            w2q = sbw.tile([P, NFT, 256], BF16, tag="w_dd")
                            nc.sync.dma_start(
                                w2q[:],
                                w2.ap()[l][:, nq * 256:(nq + 1) * 256].rearrange(
                                    "(ft p) n -> p ft n", p=P))
                            for rs in range(NRS):
                                pz = ps_mm.tile([P, 256], F32, tag="ps_mm2")
                                for ft in range(NFT):
                                    nc.tensor.matmul(pz[:], aT[:, ft, rs * P:(rs + 1) * P],
                                                     w2q[:, ft, :], start=(ft == 0),
                                                     stop=(ft == NFT - 1))
                                nc.vector.tensor_add(out=h_sb[:, rs, nq * 256:(nq + 1) * 256],
                                                     in0=h_sb[:, rs, nq * 256:(nq + 1) * 256],
                                                     in1=pz[:])
                                nc.vector.tensor_add(out=h_sb[:, rs, nq * 256:(nq + 1) * 256],
                                                     in0=h_sb[:, rs, nq * 256:(nq + 1) * 256],
                                                     in1=b2bc[:, nq * 256:(nq + 1) * 256])

                if emit == "hidden":
                    nc.sync.dma_start(h_out.ap(), h_sb[:])
                    return nc

                # ---- final layernorm -> transposed (rows stay local) ----
                with tc.tile_pool(name="ps_t", bufs=2, space="PSUM") as ps_t:
                    gf = params.tile([P, NJ], F32, tag="g1")
                    bf = params.tile([P, NJ], F32, tag="b1t")
                    nc.sync.dma_start(gf[:], lnfg.ap().rearrange("(j p) -> p j", p=P))
                    nc.sync.dma_start(bf[:], lnfb.ap().rearrange("(j p) -> p j", p=P))
                    hfT = sb1.tile([P, NJ, RPC], BF16, tag="hnT")
                    _ln_into_transposed(nc, sb3, sbw, ps_t, h_sb, range(NRS), gf, bf,
                                        ident, eps, hfT)

                # ---- full-vocab LM head for this core's 512 rows ----
                with (
                    tc.tile_pool(name="hwp", bufs=6) as hwp,
                    tc.tile_pool(name="ps_h", bufs=6, space="PSUM") as ps_h,
                ):
                    hb_sb = params.tile([P, NVT2], F32, tag="hb")
                    nc.sync.dma_start(hb_sb[:], hbs.ap())
                    for vt in range(NVT2):
                        hw_sb = hwp.tile([P, NJ, P], BF16, tag="hw_sb")
                        nc.sync.dma_start(
                            hw_sb[:],
                            hw.ap()[vt].rearrange("p (j n) -> p j n", j=NJ))
                        pl = ps_h.tile([P, RPC], F32, tag="ps_h")
                        for j in range(NJ):
                            nc.tensor.matmul(pl[:], hw_sb[:, j, :], hfT[:, j, :],
                                             start=(j == 0), stop=(j == NJ - 1))
                        lt = sb3.tile([P, RPC], BF16, tag="lt")
                        nc.vector.tensor_scalar_add(out=lt[:], in0=pl[:],
                                                    scalar1=hb_sb[:, vt:vt + 1])
                        nc.sync.dma_start(
                            logitsT.ap()[vt * P:(vt + 1) * P, :], lt[:])
    return nc


def wa_slice(w1_l, q):
    """w1[l] is [D, F]; return the q-th quarter [D, 1024] striped to [P, NJ, 1024]."""
    return w1_l[:, q * 1024:(q + 1) * 1024].rearrange("(j p) n -> p j n", p=P)


# ---------------------------------------------------------------------------
# Host side: shard inputs, run SPMD, reassemble output
# ---------------------------------------------------------------------------
def _posenc(seq_len, d_model):
    pos = np.arange(seq_len, dtype=np.float32)[:, None]
    div = np.exp(np.arange(0, d_model, 2, dtype=np.float32) * (-math.log(10000.0) / d_model))
    ang = pos * div
    pe = np.stack([np.sin(ang), np.cos(ang)], axis=-1).reshape(seq_len, d_model)
    return pe.astype(np.float32)


def make_in_maps(x, emb, ln1_g, ln1_b, wq, wk, wv, wo, ln2_g, ln2_b, w1, b1,
                 w2, b2, lnf_g, lnf_b, head_w, head_b):
    bf = ml_dtypes.bfloat16
    x = np.asarray(x)
    head_w = np.asarray(head_w, dtype=np.float32)
    head_b = np.asarray(head_b, dtype=np.float32)
    # head weights pretiled for contiguous per-partition DMA:
    # hw_tiled[vt, p, j*128+n] = head_w[j*128+p, vt*128+n]  (vocab zero-padded)
    hw_pad = np.zeros((D, VP2), dtype=bf)
    hw_pad[:, :V] = head_w.astype(bf)
    hw_tiled = np.ascontiguousarray(
        hw_pad.reshape(NJ, P, NVT2, P).transpose(2, 1, 0, 3).reshape(NVT2, P, NJ * P))
    hb_pad = np.zeros(VP2, dtype=np.float32)
    hb_pad[:V] = head_b
    hbs_t = np.ascontiguousarray(hb_pad.reshape(NVT2, P).T)
    shared = {
        "emb16": np.ascontiguousarray(np.asarray(emb, dtype=np.float16)),
        "wq": np.ascontiguousarray(np.asarray(wq, dtype=bf)),
        "wk": np.ascontiguousarray(np.asarray(wk, dtype=bf)),
        "wv": np.ascontiguousarray(np.asarray(wv, dtype=bf)),
        "wo": np.ascontiguousarray(np.asarray(wo, dtype=bf)),
        "w1": np.ascontiguousarray(np.asarray(w1, dtype=bf)),
        "w2": np.ascontiguousarray(np.asarray(w2, dtype=bf)),
        "ln1g": np.ascontiguousarray(np.asarray(ln1_g, dtype=np.float32)),
        "ln1b": np.ascontiguousarray(np.asarray(ln1_b, dtype=np.float32)),
        "ln2g": np.ascontiguousarray(np.asarray(ln2_g, dtype=np.float32)),
        "ln2b": np.ascontiguousarray(np.asarray(ln2_b, dtype=np.float32)),
        "lnfg": np.ascontiguousarray(np.asarray(lnf_g, dtype=np.float32)),
        "lnfb": np.ascontiguousarray(np.asarray(lnf_b, dtype=np.float32)),
        "b1s": np.ascontiguousarray(
            np.asarray(b1, dtype=np.float32).reshape(L, NFT, P).transpose(0, 2, 1)),
        "b2": np.ascontiguousarray(np.asarray(b2, dtype=np.float32)),
        "hw": hw_tiled,
        "hbs": hbs_t,
    }
    pe_full_arr = _posenc(S, D)
    in_maps = []
    for c in range(NC):
        b_idx, sl = c // 4, c % 4
        tok = np.asarray(x[b_idx, 512 * sl:512 * (sl + 1)], dtype=np.int32)
        idx_pa = np.ascontiguousarray(tok.reshape(NRS, P).T)
        idx_f = np.ascontiguousarray(
            np.asarray(x[b_idx], dtype=np.int32).reshape(NKC, P).T)
        pe_f = np.ascontiguousarray(pe_full_arr.reshape(NKC, P, D).transpose(1, 0, 2))
        pe_pa = np.ascontiguousarray(
            pe_full_arr[512 * sl:512 * (sl + 1)].reshape(NRS, P, D).transpose(1, 0, 2))
        kpos = (512 * (np.arange(NKC)[:, None, None] // 4)
                + 128 * (np.arange(NKC)[:, None, None] % 4)
                + np.arange(P)[None, :, None])
        rpos = 512 * sl + np.arange(RPC)[None, None, :]
        m = (kpos <= rpos).astype(bf)                        # [NKC, P, RPC]
        m_pa = np.ascontiguousarray(m.transpose(1, 0, 2))    # [P, NKC, RPC]
        in_maps.append(dict(shared, idx_pa=idx_pa, pe_pa=pe_pa, masks=m_pa,
                            idx_full=idx_f, pe_full=pe_f))
    return in_maps


def assemble(logitsT_percore):
    out = np.empty((B, S, V), dtype=np.float32)
    for c in range(NC):
        ltT = np.asarray(logitsT_percore[c][:V], dtype=np.float32).T  # [RPC, V]
        out[c // 4, 512 * (c % 4):512 * (c % 4 + 1), :] = ltT
    return out


_CACHE = {}


def get_nc(n_layers=L, emit="logits"):
    key = (n_layers, emit)
    if key not in _CACHE:
        _CACHE[key] = build(n_layers, emit)
    return _CACHE[key]


def kernel(**inputs):
    nc = get_nc()
    in_maps = make_in_maps(**inputs)
    res = bass_utils.run_bass_kernel_spmd(
        nc, in_maps, core_ids=list(range(NC)))
    return assemble([r["logitsT"] for r in res.results])


# revision 17
# speedup vs baseline: 1.3049x; 1.3049x over previous
"""MiniGPT (B=2,S=2048,D=1024,H=16,F=4096,L=4,V=50257) on 8 Trainium2 cores.

Sharding: sequence-parallel. Core c owns rows of batch c//4 at positions
[512*(c%4), 512*(c%4)+512).  Per layer each core computes QKV/attention/FFN
for its 512 rows; K/V are exchanged with one AllGather per layer within each
batch group ([[0-3],[4-7]]).  Attention is computed dense (all 16 key chunks)
with per-core 0/1 masks (preloaded once into SBUF) so the SPMD program is
uniform.  The LM head needs no collective: each core computes logits for its
own 512 rows over the FULL vocab (same FLOPs as a vocab-sharded head over all
rows), streaming the head weights from HBM in a host-pretiled layout, and
emits logits^T in bf16; the host reassembles [B,S,V] f32.  All matmuls run in
bf16 with fp32 accumulation.
"""

import json
import math

import numpy as np
import ml_dtypes

# ---------------------------------------------------------------------------
# Workaround for this container's walrus build: it can lower at most ONE sem
# wait per instruction ("Too many sync wait commands" otherwise), while Tile
# attaches several waits to one instruction.  Rewrite the BIR JSON just before
# it reaches walrus: any instruction carrying N>1 waits gets N-1 preceding
# single-wait NoOps on the same engine (identical semantics).
# ---------------------------------------------------------------------------
import concourse.bass_utils as bass_utils
import concourse.bass2jax as bass2jax

_orig_compile_bir_kernel = bass_utils.compile_bir_kernel


def _split_multi_waits(bir_json):
    m = json.loads(bir_json)
    changed = False
    for fn in m.get("functions", []):
        for blk in fn.get("blocks", []):
            new_insts = []
            for ins in blk.get("instructions", []):
                si = ins.get("sync_info")
                waits = (si or {}).get("on_wait") or []
                if len(waits) > 1:
                    changed = True
                    for i, w in enumerate(waits[:-1]):
                        new_insts.append(
                            {
                                "debug": ins.get("debug"),
                                "engine": ins["engine"],
                                "ins": [],
                                "name": f"{ins['name']}-sw{i}",
                                "opcode": "NoOp",
                                "outs": [],
                                "sync_info": {"on_update": [], "on_wait": [w]},
                                "text_hint": "split_wait",
                            }
                        )
                    si["on_wait"] = [waits[-1]]
                new_insts.append(ins)
            blk["instructions"] = new_insts
    if not changed:
        return bir_json
    return json.dumps(m).encode()


def _patched_compile_bir_kernel(bir_json, tmpdir, neff_name="file.neff"):
    if isinstance(bir_json, str):
        bir_json = bir_json.encode()
    return _orig_compile_bir_kernel(_split_multi_waits(bir_json), tmpdir, neff_name)


if bass_utils.compile_bir_kernel is _orig_compile_bir_kernel:
    bass_utils.compile_bir_kernel = _patched_compile_bir_kernel
    bass2jax.compile_bir_kernel = _patched_compile_bir_kernel

import concourse.bass as bass
import concourse.tile as tile
import concourse.mybir as mybir
from concourse.masks import make_identity

# ---------------------------------------------------------------------------
# Model / sharding constants
# ---------------------------------------------------------------------------
NC = 8                  # cores
B, S, D, H, DK, F, L, V = 2, 2048, 1024, 16, 64, 4096, 4, 50257
P = 128
RPC = 512               # rows per core
NRS = RPC // P          # 4 row subtiles
NJ = D // P             # 8 d-tiles
NHP = H // 2            # 8 head pairs
NKC = 16                # key chunks of 128 (per batch group)
NFT = F // P            # 32 ffn tiles
VP2 = 50304             # padded vocab (393 * 128)
NVT2 = VP2 // P         # 393 vocab subtiles
BF16 = mybir.dt.bfloat16
F32 = mybir.dt.float32
F16 = mybir.dt.float16
I32 = mybir.dt.int32
EXP_SCALE = 1.0 / math.sqrt(DK)


def _ln_chunk(nc, sb3, sbw, ps_t, x, g_str, b_str, ident, eps, out_T, rs):
    """LayerNorm the 128 rows in x ([P, D] f32) and write the transposed
    normalized result into out_T[:, :, rs*P:(rs+1)*P]."""
    if True:
        stats = sb3.tile([P, 2, 6], F32, tag="ln_stats")
        xg = x.rearrange("p (g d) -> p g d", g=2)
        for gi in range(2):
            nc.vector.bn_stats(out=stats[:, gi, :], in_=xg[:, gi, :])
        mv = sb3.tile([P, 2], F32, tag="ln_mv")
        nc.vector.bn_aggr(out=mv[:], in_=stats[:])
        rstd = sb3.tile([P, 1], F32, tag="ln_rstd")
        nc.scalar.activation(out=rstd[:], in_=mv[:, 1:2],
                             func=mybir.ActivationFunctionType.Sqrt,
                             bias=eps[:], scale=1.0)
        nc.vector.reciprocal(out=rstd[:], in_=rstd[:])
        hnc = sbw.tile([P, D], BF16, tag="ln_hnc")
        nc.vector.tensor_scalar(out=hnc[:], in0=x, scalar1=mv[:, 0:1], scalar2=rstd[:],
                                op0=mybir.AluOpType.subtract, op1=mybir.AluOpType.mult)
        for j in range(NJ):
            pt = ps_t.tile([P, P], BF16, tag="ps_t")
            nc.tensor.transpose(pt[:], hnc[:, j * P:(j + 1) * P], ident[:])
            nc.vector.tensor_scalar(out=out_T[:, j, rs * P:(rs + 1) * P], in0=pt[:],
                                    scalar1=g_str[:, j:j + 1], scalar2=b_str[:, j:j + 1],
                                    op0=mybir.AluOpType.mult, op1=mybir.AluOpType.add)


def _ln_into_transposed(nc, sb3, sbw, ps_t, h_sb, rs_range, g_str, b_str, ident, eps, out_T):
    for rs in rs_range:
        _ln_chunk(nc, sb3, sbw, ps_t, h_sb[:, rs, :], g_str, b_str, ident, eps, out_T, rs)


def build(n_layers=L, emit="logits"):
    """Build the SPMD Bass module.  emit: "logits" (full model) or "hidden"
    (stop after n_layers, output h [P, NRS, D] f32 for debugging)."""
    nc = bass.Bass(num_devices=NC)

    # ---- inputs (per core) ----
    emb16 = nc.dram_tensor("emb16", [V, D], F16, kind="ExternalInput")
    idx_pa = nc.dram_tensor("idx_pa", [P, NRS], I32, kind="ExternalInput")
    idx_full = nc.dram_tensor("idx_full", [P, NKC], I32, kind="ExternalInput")
    pe_full = nc.dram_tensor("pe_full", [P, NKC, D], F32, kind="ExternalInput")
    pe_pa = nc.dram_tensor("pe_pa", [P, NRS, D], F32, kind="ExternalInput")
    wq = nc.dram_tensor("wq", [L, D, D], BF16, kind="ExternalInput")
    wk = nc.dram_tensor("wk", [L, D, D], BF16, kind="ExternalInput")
    wv = nc.dram_tensor("wv", [L, D, D], BF16, kind="ExternalInput")
    wo = nc.dram_tensor("wo", [L, D, D], BF16, kind="ExternalInput")
    w1 = nc.dram_tensor("w1", [L, D, F], BF16, kind="ExternalInput")
    w2 = nc.dram_tensor("w2", [L, F, D], BF16, kind="ExternalInput")
    ln1g = nc.dram_tensor("ln1g", [L, D], F32, kind="ExternalInput")
    ln1b = nc.dram_tensor("ln1b", [L, D], F32, kind="ExternalInput")
    ln2g = nc.dram_tensor("ln2g", [L, D], F32, kind="ExternalInput")
    ln2b = nc.dram_tensor("ln2b", [L, D], F32, kind="ExternalInput")
    lnfg = nc.dram_tensor("lnfg", [D], F32, kind="ExternalInput")
    lnfb = nc.dram_tensor("lnfb", [D], F32, kind="ExternalInput")
    b1s = nc.dram_tensor("b1s", [L, P, NFT], F32, kind="ExternalInput")
    b2 = nc.dram_tensor("b2", [L, D], F32, kind="ExternalInput")
    masks = nc.dram_tensor("masks", [P, NKC, RPC], BF16, kind="ExternalInput")
    hw = nc.dram_tensor("hw", [NVT2, P, NJ * P], BF16, kind="ExternalInput")
    hbs = nc.dram_tensor("hbs", [P, NVT2], F32, kind="ExternalInput")

    if emit == "hidden":
        h_out = nc.dram_tensor("h_out", [P, NRS, D], F32, kind="ExternalOutput")
    else:
        logitsT = nc.dram_tensor("logitsT", [VP2, RPC], BF16, kind="ExternalOutput")

    kv_groups = [[0, 1, 2, 3], [4, 5, 6, 7]]

    with tile.TileContext(nc) as tc:
        with (
            tc.tile_pool(name="singles", bufs=1) as singles,
            tc.tile_pool(name="h", bufs=1) as hpool,
            tc.tile_pool(name="params", bufs=1) as params,
            tc.tile_pool(name="sbw", bufs=2) as sbw,
            tc.tile_pool(name="sb3", bufs=3) as sb3,
            tc.tile_pool(name="dram", bufs=3, space="DRAM") as dram,
        ):
            ident = singles.tile([P, P], BF16)
            make_identity(nc, ident[:])
            eps = singles.tile([P, 1], F32)
            nc.vector.memset(eps[:], 1e-5)
            ones64 = singles.tile([1, 64], F32)
            nc.vector.memset(ones64[:], 1.0)
            h_sb = hpool.tile([P, NRS, D], F32)

            # ---- embedding gather + positional encoding ----
            # (tiny index DMAs go first so the gathers are not queued behind
            # the 2MB mask load)
            idx_sb = sb3.tile([P, NRS], I32, tag="idx")
            nc.sync.dma_start(idx_sb[:], idx_pa.ap())
            mask_sb = singles.tile([P, NKC, RPC], BF16)
            nc.sync.dma_start(mask_sb[:], masks.ap())
            for rs in range(NRS):
                eg = sbw.tile([P, D], F16, tag="embg")
                nc.gpsimd.indirect_dma_start(
                    out=eg[:], out_offset=None, in_=emb16.ap(),
                    in_offset=bass.IndirectOffsetOnAxis(ap=idx_sb[:, rs:rs + 1], axis=0),
                )
                pe = sbw.tile([P, D], F32, tag="peg")
                nc.sync.dma_start(pe[:], pe_pa.ap()[:, rs, :])
                nc.vector.tensor_copy(out=h_sb[:, rs, :], in_=eg[:])
                nc.vector.tensor_add(out=h_sb[:, rs, :], in0=h_sb[:, rs, :], in1=pe[:])

            with tc.tile_pool(name="sb1", bufs=1) as sb1:
                # ---- layer 0: every core computes K/V for ALL 2048 rows of
                # its batch straight from the (replicated) embeddings — no
                # AllGather needed for layer 0. ----
                kv0 = dram.tile([4, 2, P, NRS * D], BF16, tag="kv_all")
                g10 = params.tile([P, NJ], F32, tag="g1")
                b10 = params.tile([P, NJ], F32, tag="b1t")
                nc.sync.dma_start(g10[:], ln1g.ap()[0].rearrange("(j p) -> p j", p=P))
                nc.sync.dma_start(b10[:], ln1b.ap()[0].rearrange("(j p) -> p j", p=P))
                idxf_sb = sb3.tile([P, NKC], I32, tag="idx")
                nc.sync.dma_start(idxf_sb[:], idx_full.ap())
                with (
                    tc.tile_pool(name="ps_mm0", bufs=3, space="PSUM") as ps_mm0,
                    tc.tile_pool(name="ps_t0", bufs=2, space="PSUM") as ps_t0,
                ):
                    wk0 = sbw.tile([P, NJ, D], BF16, tag="w_dd")
                    nc.sync.dma_start(wk0[:], wk.ap()[0].rearrange("(j p) n -> p j n", p=P))
                    wv0 = sbw.tile([P, NJ, D], BF16, tag="w_dd")
                    nc.sync.dma_start(wv0[:], wv.ap()[0].rearrange("(j p) n -> p j n", p=P))
                    for b4 in range(4):
                        hnTg = sb1.tile([P, NJ, RPC], BF16, tag="hnT")
                        for rs in range(NRS):
                            rc = b4 * NRS + rs
                            eg = sbw.tile([P, D], F16, tag="embg")
                            nc.gpsimd.indirect_dma_start(
                                out=eg[:], out_offset=None, in_=emb16.ap(),
                                in_offset=bass.IndirectOffsetOnAxis(
                                    ap=idxf_sb[:, rc:rc + 1], axis=0),
                            )
                            pef = sbw.tile([P, D], F32, tag="peg")
                            nc.sync.dma_start(pef[:], pe_full.ap()[:, rc, :])
                            hch = sb3.tile([P, D], F32, tag="hch")
                            nc.vector.tensor_copy(out=hch[:], in_=eg[:])
                            nc.vector.tensor_add(out=hch[:], in0=hch[:], in1=pef[:])
                            _ln_chunk(nc, sb3, sbw, ps_t0, hch[:], g10, b10,
                                      ident, eps, hnTg, rs)
                        ktg = sb1.tile([P, NHP, RPC], BF16, tag="kt")
                        for hp in range(NHP):
                            pq = ps_mm0.tile([P, RPC], F32, tag="ps_mm", bufs=4)
                            for j in range(NJ):
                                nc.tensor.matmul(pq[:], wk0[:, j, hp * P:(hp + 1) * P],
                                                 hnTg[:, j, :], start=(j == 0),
                                                 stop=(j == NJ - 1))
                            nc.scalar.activation(out=ktg[:, hp, :], in_=pq[:],
                                                 func=mybir.ActivationFunctionType.Copy)
                        nc.sync.dma_start(kv0[b4, 0], ktg[:].rearrange("p a b -> p (a b)"))
                        vg = sb1.tile([P, NRS, D], BF16, tag="vown")
                        for rs in range(NRS):
                            for nh in range(2):
                                pv = ps_mm0.tile([P, RPC], F32, tag="ps_mm", bufs=4)
                                for j in range(NJ):
                                    nc.tensor.matmul(pv[:], hnTg[:, j, rs * P:(rs + 1) * P],
                                                     wv0[:, j, nh * 512:(nh + 1) * 512],
                                                     start=(j == 0), stop=(j == NJ - 1))
                                nc.scalar.activation(out=vg[:, rs, nh * 512:(nh + 1) * 512],
                                                     in_=pv[:],
                                                     func=mybir.ActivationFunctionType.Copy)
                        nc.sync.dma_start(kv0[b4, 1], vg[:].rearrange("p a b -> p (a b)"))
                    # own rows: LN1 + Q projection for layer 0
                    hnT0 = sb1.tile([P, NJ, RPC], BF16, tag="hnT")
                    _ln_into_transposed(nc, sb3, sbw, ps_t0, h_sb, range(NRS), g10, b10,
                                        ident, eps, hnT0)
                    qt0 = sb1.tile([P, NHP, RPC], BF16, tag="qt")
                    wq0 = sbw.tile([P, NJ, D], BF16, tag="w_dd")
                    nc.sync.dma_start(wq0[:], wq.ap()[0].rearrange("(j p) n -> p j n", p=P))
                    for hp in range(NHP):
                        pq = ps_mm0.tile([P, RPC], F32, tag="ps_mm", bufs=4)
                        for j in range(NJ):
                            nc.tensor.matmul(pq[:], wq0[:, j, hp * P:(hp + 1) * P],
                                             hnT0[:, j, :], start=(j == 0), stop=(j == NJ - 1))
                        nc.scalar.activation(out=qt0[:, hp, :], in_=pq[:],
                                             func=mybir.ActivationFunctionType.Copy)

                for l in range(n_layers):
                    # ---- layer parameter tiles ----
                    g2 = params.tile([P, NJ], F32, tag="g2")
                    b2t = params.tile([P, NJ], F32, tag="b2t")
                    nc.sync.dma_start(g2[:], ln2g.ap()[l].rearrange("(j p) -> p j", p=P))
                    nc.sync.dma_start(b2t[:], ln2b.ap()[l].rearrange("(j p) -> p j", p=P))
                    b1v = params.tile([P, NFT], F32, tag="b1v")
                    nc.sync.dma_start(b1v[:], b1s.ap()[l])
                    b2bc = params.tile([P, D], F32, tag="b2bc")
                    nc.sync.dma_start(
                        b2bc[:],
                        bass.AP(tensor=b2.ap().tensor, offset=l * D, ap=[[0, P], [1, D]]),
                    )

                    if l == 0:
                        kv_all = kv0
                        qt_sb = qt0
                    else:
                        g1 = params.tile([P, NJ], F32, tag="g1")
                        b1t = params.tile([P, NJ], F32, tag="b1t")
                        nc.sync.dma_start(g1[:], ln1g.ap()[l].rearrange("(j p) -> p j", p=P))
                        nc.sync.dma_start(b1t[:], ln1b.ap()[l].rearrange("(j p) -> p j", p=P))

                        kv_in = dram.tile([2, P, NRS * D], BF16, tag="kv_in")
                        kv_all = dram.tile([4, 2, P, NRS * D], BF16, tag="kv_all")

                        with (
                            tc.tile_pool(name="ps_mm", bufs=3, space="PSUM") as ps_mm,
                            tc.tile_pool(name="ps_t", bufs=2, space="PSUM") as ps_t,
                        ):
                            # ---- ln1 -> hnT ----
                            hnT = sb1.tile([P, NJ, RPC], BF16, tag="hnT")
                            _ln_into_transposed(nc, sb3, sbw, ps_t, h_sb, range(NRS), g1, b1t,
                                                ident, eps, hnT)

                            # ---- K and V projections first (feed the AllGather) ----
                            kt_sb = sb1.tile([P, NHP, RPC], BF16, tag="kt")
                            wk_sb = sbw.tile([P, NJ, D], BF16, tag="w_dd")
                            nc.sync.dma_start(wk_sb[:], wk.ap()[l].rearrange("(j p) n -> p j n", p=P))
                            for hp in range(NHP):
                                pq = ps_mm.tile([P, RPC], F32, tag="ps_mm", bufs=4)
                                for j in range(NJ):
                                    nc.tensor.matmul(pq[:], wk_sb[:, j, hp * P:(hp + 1) * P],
                                                     hnT[:, j, :], start=(j == 0), stop=(j == NJ - 1))
                                nc.scalar.activation(out=kt_sb[:, hp, :], in_=pq[:],
                                                     func=mybir.ActivationFunctionType.Copy)
                            nc.sync.dma_start(kv_in[0], kt_sb[:].rearrange("p a b -> p (a b)"))

                            v_sb = sb1.tile([P, NRS, D], BF16, tag="vown")
                            wv_sb = sbw.tile([P, NJ, D], BF16, tag="w_dd")
                            nc.sync.dma_start(wv_sb[:], wv.ap()[l].rearrange("(j p) n -> p j n", p=P))
                            for rs in range(NRS):
                                for nh in range(2):
                                    pv = ps_mm.tile([P, RPC], F32, tag="ps_mm", bufs=4)
                                    for j in range(NJ):
                                        nc.tensor.matmul(pv[:], hnT[:, j, rs * P:(rs + 1) * P],
                                                         wv_sb[:, j, nh * 512:(nh + 1) * 512],
                                                         start=(j == 0), stop=(j == NJ - 1))
                                    nc.scalar.activation(out=v_sb[:, rs, nh * 512:(nh + 1) * 512],
                                                         in_=pv[:],
                                                         func=mybir.ActivationFunctionType.Copy)
                            nc.sync.dma_start(kv_in[1], v_sb[:].rearrange("p a b -> p (a b)"))

                        nc.gpsimd.collective_compute(
                            "AllGather", mybir.AluOpType.bypass, replica_groups=kv_groups,
                            ins=[kv_in.opt()], outs=[kv_all.opt()],
                        )

                        with (
                            tc.tile_pool(name="ps_mm", bufs=3, space="PSUM") as ps_mm,
                        ):
                            # ---- Q projection (overlaps the AllGather) ----
                            qt_sb = sb1.tile([P, NHP, RPC], BF16, tag="qt")
                            wq_sb = sbw.tile([P, NJ, D], BF16, tag="w_dd")
                            nc.sync.dma_start(wq_sb[:], wq.ap()[l].rearrange("(j p) n -> p j n", p=P))
                            for hp in range(NHP):
                                pq = ps_mm.tile([P, RPC], F32, tag="ps_mm", bufs=4)
                                for j in range(NJ):
                                    nc.tensor.matmul(pq[:], wq_sb[:, j, hp * P:(hp + 1) * P],
                                                     hnT[:, j, :], start=(j == 0), stop=(j == NJ - 1))
                                nc.scalar.activation(out=qt_sb[:, hp, :], in_=pq[:],
                                                     func=mybir.ActivationFunctionType.Copy)

                    kv_k = kv_all[:].rearrange("g t p (hp r) -> g t p hp r", hp=NHP)
                    kv_v = kv_all[:].rearrange("g t p (rs hh d) -> g t p rs hh d", rs=NRS, hh=H)

                    # ---- attention (dense over 16 key chunks, masked) ----
                    attnT = sb1.tile([P, NHP, RPC], BF16, tag="attnT")
                    with (
                        tc.tile_pool(name="ps_s", bufs=2, space="PSUM") as ps_s,
                        tc.tile_pool(name="ps_pv", bufs=4, space="PSUM") as ps_pv,
                        tc.tile_pool(name="ps_bc", bufs=1, space="PSUM") as ps_bc,
                    ):
                        for hg in range(4):
                            pvs = [ps_pv.tile([65, RPC], F32, tag="ps_pv", name=f"pv{hg}_{i}") for i in range(4)]
                            for kc in range(NKC):
                                gr, rs = kc // 4, kc % 4
                                ktt = sb3.tile([P, 2, P], BF16, tag="ktt", bufs=6)
                                nc.sync.dma_start(
                                    ktt[:], kv_k[gr, 0, :, 2 * hg:2 * hg + 2, rs * P:(rs + 1) * P])
                                vat = sb3.tile([P, 4, 65], BF16, tag="vat", bufs=6)
                                nc.vector.memset(vat[:, :, 64:65], 1.0)
                                nc.sync.dma_start(
                                    vat[:, :, 0:64], kv_v[gr, 1, :, rs, 4 * hg:4 * hg + 4, :])
                                for hi in range(4):
                                    h_ = 4 * hg + hi
                                    hp, o = h_ // 2, (h_ % 2) * 64
                                    pss = ps_s.tile([P, RPC], F32, tag="ps_s")
                                    nc.tensor.matmul(pss[:], ktt[o:o + 64, hi // 2, :],
                                                     qt_sb[o:o + 64, hp, :], start=True, stop=True)
                                    et = sb3.tile([P, RPC], BF16, tag="et", bufs=4)
                                    nc.scalar.activation(out=et[:], in_=pss[:],
                                                         func=mybir.ActivationFunctionType.Exp,
                                                         scale=EXP_SCALE)
                                    nc.vector.tensor_mul(out=et[:], in0=et[:], in1=mask_sb[:, kc, :])
                                    nc.tensor.matmul(pvs[hi][:], vat[:, hi, :], et[:],
                                                     start=(kc == 0), stop=(kc == NKC - 1))
                            for hi in range(4):
                                h_ = 4 * hg + hi
                                hp, o = h_ // 2, (h_ % 2) * 64
                                rec = sb3.tile([1, RPC], F32, tag="rec")
                                nc.vector.reciprocal(out=rec[:], in_=pvs[hi][64:65, :])
                                pbc = ps_bc.tile([64, RPC], F32, tag="ps_bc")
                                nc.tensor.matmul(pbc[:], ones64[:], rec[:], start=True, stop=True)
                                bcs = sb3.tile([64, RPC], F32, tag="bcs")
                                nc.scalar.activation(out=bcs[:], in_=pbc[:],
                                                     func=mybir.ActivationFunctionType.Copy)
                                nc.vector.tensor_mul(out=attnT[o:o + 64, hp, :],
                                                     in0=pvs[hi][0:64, :], in1=bcs[:])

                    with (
                        tc.tile_pool(name="ps_mm", bufs=3, space="PSUM") as ps_mm,
                        tc.tile_pool(name="ps_t", bufs=2, space="PSUM") as ps_t,
                    ):
                        # ---- attn output projection + residual ----
                        wo_sb = sbw.tile([P, NJ, D], BF16, tag="w_dd")
                        nc.sync.dma_start(wo_sb[:], wo.ap()[l].rearrange("(j p) n -> p j n", p=P))
                        for rs in range(NRS):
                            for nh in range(2):
                                po = ps_mm.tile([P, RPC], F32, tag="ps_mm", bufs=4)
                                for j in range(NJ):
                                    nc.tensor.matmul(po[:], attnT[:, j, rs * P:(rs + 1) * P],
                                                     wo_sb[:, j, nh * 512:(nh + 1) * 512],
                                                     start=(j == 0), stop=(j == NJ - 1))
                                nc.vector.tensor_add(out=h_sb[:, rs, nh * 512:(nh + 1) * 512],
                                                     in0=h_sb[:, rs, nh * 512:(nh + 1) * 512],
                                                     in1=po[:])

                        # ---- FFN ----
                        hnT2 = sb1.tile([P, NJ, RPC], BF16, tag="hnT")
                        _ln_into_transposed(nc, sb3, sbw, ps_t, h_sb, range(NRS), g2, b2t,
                                            ident, eps, hnT2)
                        aT = sb1.tile([P, NFT, RPC], BF16, tag="aT")
                        for q in range(4):
                            w1q = sbw.tile([P, NJ, D], BF16, tag="w_dd")
                            nc.sync.dma_start(
                                w1q[:],
                                wa_slice(w1.ap()[l], q))
                            for fl in range(8):
                                ft = q * 8 + fl
                                pa = ps_mm.tile([P, RPC], F32, tag="ps_mm", bufs=4)
                                for j in range(NJ):
                                    nc.tensor.matmul(pa[:], w1q[:, j, fl * P:(fl + 1) * P],
                                                     hnT2[:, j, :], start=(j == 0), stop=(j == NJ - 1))
                                nc.scalar.activation(out=aT[:, ft, :], in_=pa[:],
                                                     func=mybir.ActivationFunctionType.Relu,
                                                     bias=b1v[:, ft:ft + 1], scale=1.0)
                        for nq in range(4):
                            w2q = sbw.tile([P, NFT, 256], BF16, tag="w_dd")
                            nc.sync.dma_start(
                                w2q[:],
                                w2.ap()[l][:, nq * 256:(nq + 1) * 256].rearrange(
                                    "(ft p) n -> p ft n", p=P))
                            for rs in range(NRS):
                                pz = ps_mm.tile([P, 256], F32, tag="ps_mm2", bufs=2)
                                for ft in range(NFT):
                                    nc.tensor.matmul(pz[:], aT[:, ft, rs * P:(rs + 1) * P],
                                                     w2q[:, ft, :], start=(ft == 0),
                                                     stop=(ft == NFT - 1))
                                nc.vector.tensor_add(out=h_sb[:, rs, nq * 256:(nq + 1) * 256],
                                                     in0=h_sb[:, rs, nq * 256:(nq + 1) * 256],
                                                     in1=pz[:])
                                nc.vector.tensor_add(out=h_sb[:, rs, nq * 256:(nq + 1) * 256],
                                                     in0=h_sb[:, rs, nq * 256:(nq + 1) * 256],
                                                     in1=b2bc[:, nq * 256:(nq + 1) * 256])

                if emit == "hidden":
                    nc.sync.dma_start(h_out.ap(), h_sb[:])
                    return nc

                # ---- final layernorm -> transposed (rows stay local) ----
                with tc.tile_pool(name="ps_t", bufs=2, space="PSUM") as ps_t:
                    gf = params.tile([P, NJ], F32, tag="g1")
                    bf = params.tile([P, NJ], F32, tag="b1t")
                    nc.sync.dma_start(gf[:], lnfg.ap().rearrange("(j p) -> p j", p=P))
                    nc.sync.dma_start(bf[:], lnfb.ap().rearrange("(j p) -> p j", p=P))
                    hfT = sb1.tile([P, NJ, RPC], BF16, tag="hnT")
                    _ln_into_transposed(nc, sb3, sbw, ps_t, h_sb, range(NRS), gf, bf,
                                        ident, eps, hfT)

                # ---- full-vocab LM head for this core's 512 rows ----
                with (
                    tc.tile_pool(name="hwp", bufs=5) as hwp,
                    tc.tile_pool(name="ps_h", bufs=6, space="PSUM") as ps_h,
                ):
                    hb_sb = params.tile([P, NVT2], F32, tag="hb")
                    nc.sync.dma_start(hb_sb[:], hbs.ap())
                    for vt in range(NVT2):
                        hw_sb = hwp.tile([P, NJ, P], BF16, tag="hw_sb")
                        nc.sync.dma_start(
                            hw_sb[:],
                            hw.ap()[vt].rearrange("p (j n) -> p j n", j=NJ))
                        pl = ps_h.tile([P, RPC], F32, tag="ps_h")
                        for j in range(NJ):
                            nc.tensor.matmul(pl[:], hw_sb[:, j, :], hfT[:, j, :],
                                             start=(j == 0), stop=(j == NJ - 1))
                        lt = sb3.tile([P, RPC], BF16, tag="lt")
                        nc.vector.tensor_scalar_add(out=lt[:], in0=pl[:],
                                                    scalar1=hb_sb[:, vt:vt + 1])
                        nc.sync.dma_start(
                            logitsT.ap()[vt * P:(vt + 1) * P, :], lt[:])
    return nc


def wa_slice(w1_l, q):
    """w1[l] is [D, F]; return the q-th quarter [D, 1024] striped to [P, NJ, 1024]."""
    return w1_l[:, q * 1024:(q + 1) * 1024].rearrange("(j p) n -> p j n", p=P)


# ---------------------------------------------------------------------------
# Host side: shard inputs, run SPMD, reassemble output
# ---------------------------------------------------------------------------
def _posenc(seq_len, d_model):
    pos = np.arange(seq_len, dtype=np.float32)[:, None]
    div = np.exp(np.arange(0, d_model, 2, dtype=np.float32) * (-math.log(10000.0) / d_model))
    ang = pos * div
    pe = np.stack([np.sin(ang), np.cos(ang)], axis=-1).reshape(seq_len, d_model)
    return pe.astype(np.float32)


def make_in_maps(x, emb, ln1_g, ln1_b, wq, wk, wv, wo, ln2_g, ln2_b, w1, b1,
                 w2, b2, lnf_g, lnf_b, head_w, head_b):
    bf = ml_dtypes.bfloat16
    x = np.asarray(x)
    head_w = np.asarray(head_w, dtype=np.float32)
    head_b = np.asarray(head_b, dtype=np.float32)
    # head weights pretiled for contiguous per-partition DMA:
    # hw_tiled[vt, p, j*128+n] = head_w[j*128+p, vt*128+n]  (vocab zero-padded)
    hw_pad = np.zeros((D, VP2), dtype=bf)
    hw_pad[:, :V] = head_w.astype(bf)
    hw_tiled = np.ascontiguousarray(
        hw_pad.reshape(NJ, P, NVT2, P).transpose(2, 1, 0, 3).reshape(NVT2, P, NJ * P))
    hb_pad = np.zeros(VP2, dtype=np.float32)
    hb_pad[:V] = head_b
    hbs_t = np.ascontiguousarray(hb_pad.reshape(NVT2, P).T)
    shared = {
        "emb16": np.ascontiguousarray(np.asarray(emb, dtype=np.float16)),
        "wq": np.ascontiguousarray(np.asarray(wq, dtype=bf)),
        "wk": np.ascontiguousarray(np.asarray(wk, dtype=bf)),
        "wv": np.ascontiguousarray(np.asarray(wv, dtype=bf)),
        "wo": np.ascontiguousarray(np.asarray(wo, dtype=bf)),
        "w1": np.ascontiguousarray(np.asarray(w1, dtype=bf)),
        "w2": np.ascontiguousarray(np.asarray(w2, dtype=bf)),
        "ln1g": np.ascontiguousarray(np.asarray(ln1_g, dtype=np.float32)),
        "ln1b": np.ascontiguousarray(np.asarray(ln1_b, dtype=np.float32)),
        "ln2g": np.ascontiguousarray(np.asarray(ln2_g, dtype=np.float32)),
        "ln2b": np.ascontiguousarray(np.asarray(ln2_b, dtype=np.float32)),
        "lnfg": np.ascontiguousarray(np.asarray(lnf_g, dtype=np.float32)),
        "lnfb": np.ascontiguousarray(np.asarray(lnf_b, dtype=np.float32)),
        "b1s": np.ascontiguousarray(
            np.asarray(b1, dtype=np.float32).reshape(L, NFT, P).transpose(0, 2, 1)),
        "b2": np.ascontiguousarray(np.asarray(b2, dtype=np.float32)),
        "hw": hw_tiled,
        "hbs": hbs_t,
    }
    pe_full_arr = _posenc(S, D)
    in_maps = []
    for c in range(NC):
        b_idx, sl = c // 4, c % 4
        tok = np.asarray(x[b_idx, 512 * sl:512 * (sl + 1)], dtype=np.int32)
        idx_pa = np.ascontiguousarray(tok.reshape(NRS, P).T)
        idx_f = np.ascontiguousarray(
            np.asarray(x[b_idx], dtype=np.int32).reshape(NKC, P).T)
        pe_f = np.ascontiguousarray(pe_full_arr.reshape(NKC, P, D).transpose(1, 0, 2))
        pe_pa = np.ascontiguousarray(
            pe_full_arr[512 * sl:512 * (sl + 1)].reshape(NRS, P, D).transpose(1, 0, 2))
        kpos = (512 * (np.arange(NKC)[:, None, None] // 4)
                + 128 * (np.arange(NKC)[:, None, None] % 4)
                + np.arange(P)[None, :, None])
        rpos = 512 * sl + np.arange(RPC)[None, None, :]
        m = (kpos <= rpos).astype(bf)                        # [NKC, P, RPC]
        m_pa = np.ascontiguousarray(m.transpose(1, 0, 2))    # [P, NKC, RPC]
        in_maps.append(dict(shared, idx_pa=idx_pa, pe_pa=pe_pa, masks=m_pa,
                            idx_full=idx_f, pe_full=pe_f))
    return in_maps


def assemble(logitsT_percore):
    out = np.empty((B, S, V), dtype=np.float32)
    for c in range(NC):
        ltT = np.asarray(logitsT_percore[c][:V], dtype=np.float32).T  # [RPC, V]
        out[c // 4, 512 * (c % 4):512 * (c % 4 + 1), :] = ltT
    return out


_CACHE = {}


def get_nc(n_layers=L, emit="logits"):
    key = (n_layers, emit)
    if key not in _CACHE:
        _CACHE[key] = build(n_layers, emit)
    return _CACHE[key]


def kernel(**inputs):
    nc = get_nc()
    in_maps = make_in_maps(**inputs)
    res = bass_utils.run_bass_kernel_spmd(
        nc, in_maps, core_ids=list(range(NC)))
    return assemble([r["logitsT"] for r in res.results])
